# revision 10
# baseline (speedup 1.0000x reference)
"""Neural CDE kernel for Trainium2 (8 NeuronCores, data-parallel over batch).

Problem shapes (hardcoded per contract): B=512, T=1024, D=8, H=64, W=128.

Host side: knot index / frac from ts (exact fp32 accumulation semantics),
spline derivative dX, initial MLP y0, and folding of dt plus the
tanh(z) = 1 - 2*sigmoid(-2z) rewrite into a padded dX9 tensor.

Device side (per core, 64 samples, scan fully unrolled):
  p1 = Wf0 @ y            (PE, weight-stationary)
  h1 = ln(1 + exp(p1+b0)) (ACT Exp + Ln(bias=1))   [natural_log_exp set]
  p2 = Wf1 @ h1           (PE)
  h2 = ln(1 + exp(p2+b1)) (ACT)
  z  = Wf2 @ h2 + b2      (PE, data-stationary, + K=1 ones-matmul for bias)
  S  = sigmoid(-2z) = exp(-ln(1+exp(2z)))          (ACT x3)
  q[s,h] = sum_d S9[s,(h,d)] * dX9[s,k,d]          (DVE mul + grouped reduce)
           where S9 has a constant-1 column at d=8 and
           dX9[...,d<8] = -2*dt*dX, dX9[...,8] = dt*sum_d dX
           => q = dt * sum_d tanh(z_d) * dX_d
  y += q^T                (PE transpose + DVE add)
  ro[:,k] = y^T @ Wl      (PE, N=1 matmul into accumulating PSUM bank)
Final: sigmoid via the same exp/ln chain, DMA out.
"""

import numpy as np

B, T, D, H, W = 512, 1024, 8, 64, 128
NCORES = 8
S = B // NCORES  # samples per core = 64
D9 = D + 1       # padded derivative cols


# ----------------------------------------------------------------- host math
def _host_precompute(ts, cd, cc, cb, ca, Wi0, bi0, Wi1, bi1, Wi2, bi2):
    f32 = np.float32
    ts = np.asarray(ts, f32)
    dt = (ts[:, 1] - ts[:, 0]).astype(f32)  # (B,)

    # t0 series: t0_{k+1} = t0_k + dt accumulated in fp32 (cumsum is sequential)
    incs = np.concatenate([ts[:, :1], np.tile(dt[:, None], (1, T - 1))], axis=1)
    t0 = np.cumsum(incs, axis=1, dtype=f32)  # (B, T)

    # knot index + frac per row (searchsorted 'right' like the oracle)
    idx = np.empty((B, T), np.int64)
    for b in range(B):
        idx[b] = np.searchsorted(ts[b], t0[b], side="right") - 1
    idx = np.clip(idx, 0, T - 2)
    frac = (t0 - np.take_along_axis(ts, idx, axis=1)).astype(f32)  # (B, T)

    rows = np.arange(B)[:, None]
    cbg = cb[rows, idx]  # (B, T, D)
    ccg = cc[rows, idx]
    cdg = cd[rows, idx]
    fr = frac[:, :, None]
    dX = (cbg + fr * (f32(2.0) * ccg + f32(3.0) * fr * cdg)).astype(f32)

    dtb = dt[:, None, None]
    dX9 = np.empty((B, T, D9), f32)
    dX9[:, :, :D] = f32(-2.0) * dtb * dX
    dX9[:, :, D] = (dtb[:, :, 0] * dX.sum(axis=2)).astype(f32)

    # initial MLP (relu hidden): y0 = Wi2 @ relu(Wi1 @ relu(Wi0 @ a0 + bi0) + bi1) + bi2
    a0 = np.asarray(ca, f32)[:, 0, :]  # (B, D)
    hh = np.maximum(a0 @ np.asarray(Wi0, f32).T + bi0, 0)
    hh = np.maximum(hh @ np.asarray(Wi1, f32).T + bi1, 0)
    y0 = (hh @ np.asarray(Wi2, f32).T + bi2).astype(f32)  # (B, H)
    return dX9, y0


# --------------------------------------------------------------- bass kernel
def _build_kernel(bl_val):
    import concourse.bass as bass
    import concourse.bacc as bacc
    import concourse.mybir as mybir
    from concourse.tile import TileContext

    f32 = mybir.dt.float32
    AF = mybir.ActivationFunctionType
    ALU = mybir.AluOpType

    nc = bacc.Bacc("TRN2")

    # DRAM I/O (per-core shapes)
    d_w0t = nc.dram_tensor("w0t", [H, W], f32, kind="ExternalInput")      # Wf0^T
    d_w1t = nc.dram_tensor("w1t", [W, W], f32, kind="ExternalInput")      # Wf1^T
    d_w2t = nc.dram_tensor("w2t", [W, H * D], f32, kind="ExternalInput")  # Wf2^T
    d_wlt = nc.dram_tensor("wlt", [H, 1], f32, kind="ExternalInput")      # Wl^T
    d_b0 = nc.dram_tensor("b0", [W, 1], f32, kind="ExternalInput")
    d_b1 = nc.dram_tensor("b1", [W, 1], f32, kind="ExternalInput")
    d_b2 = nc.dram_tensor("b2", [1, H * D], f32, kind="ExternalInput")
    d_ones = nc.dram_tensor("ones1", [1, S], f32, kind="ExternalInput")
    d_ident = nc.dram_tensor("ident", [S, S], f32, kind="ExternalInput")
    d_dx9 = nc.dram_tensor("dx9", [S, T * D9], f32, kind="ExternalInput")
    d_y0t = nc.dram_tensor("y0t", [H, S], f32, kind="ExternalInput")
    d_out = nc.dram_tensor("out", [S, T], f32, kind="ExternalOutput")

    UNROLL = 16
    assert T % UNROLL == 0

    with TileContext(nc) as tc:
        with (
            tc.tile_pool(name="const", bufs=1) as cpool,
            tc.tile_pool(name="state", bufs=1) as spool,
            tc.tile_pool(name="work", bufs=2) as wpool,
            tc.tile_pool(name="ps", bufs=2, space="PSUM") as ppool,
            tc.tile_pool(name="ps1", bufs=1, space="PSUM") as p1pool,
        ):
            # constants
            w0t = cpool.tile([H, W], f32, tag="w0t")
            w1t = cpool.tile([W, W], f32, tag="w1t")
            w2t = cpool.tile([W, H * D], f32, tag="w2t")
            wlt = cpool.tile([H, 1], f32, tag="wlt")
            b0 = cpool.tile([W, 1], f32, tag="b0")
            b1 = cpool.tile([W, 1], f32, tag="b1")
            b2 = cpool.tile([1, H * D], f32, tag="b2")
            ones1 = cpool.tile([1, S], f32, tag="ones1")
            ident = cpool.tile([S, S], f32, tag="ident")
            dx9 = cpool.tile([S, T * D9], f32, tag="dx9")
            for dst, src in [
                (w0t, d_w0t), (w1t, d_w1t), (w2t, d_w2t), (wlt, d_wlt),
                (b0, d_b0), (b1, d_b1), (b2, d_b2), (ones1, d_ones),
                (ident, d_ident), (dx9, d_dx9),
            ]:
                nc.gpsimd.dma_start(dst[:], src[:])

            # state
            y = spool.tile([H, S], f32, tag="y")  # (h, s)
            nc.gpsimd.dma_start(y[:], d_y0t[:])
            # S9 double buffer, const-1 column at d=8
            s9 = [
                spool.tile([S, H * D9], f32, tag=f"s9_{i}", name=f"s9_{i}")
                for i in range(2)
            ]
            for t_ in s9:
                v = t_[:].rearrange("s (h d) -> s h d", d=D9)
                nc.vector.memset(v[:, :, D : D + 1], 1.0)

            ro_sb = spool.tile([S, T], f32, tag="ro_sb")
            ro_ps = p1pool.tile([S, UNROLL], f32, tag="ro_ps")

            # Constants settle before any compute touches them: a matmul
            # (S3_LW struct) cannot carry multiple HWDGE sem waits.
            tc.strict_bb_all_engine_barrier()

            with tc.For_i(0, T // UNROLL, 1) as iv:
              ibase = iv * (UNROLL * D9)
              for j in range(UNROLL):
                k = j  # static within the unrolled body
                s9k = s9[k % 2]
                # ---- mm1: p1 = Wf0 @ y  -> (W, S)
                p1 = ppool.tile([W, S], f32, tag="p12")
                nc.tensor.matmul(p1[:], w0t[:], y[:], start=True, stop=True)
                # ---- softplus 1 (with bias b0 folded into Exp)
                u1 = wpool.tile([W, S], f32, tag="u1")
                h1 = wpool.tile([W, S], f32, tag="h1")
                nc.scalar.activation(u1[:], p1[:], AF.Exp, bias=b0[:])
                nc.scalar.activation(h1[:], u1[:], AF.Ln, bias=1.0)
                # ---- mm2
                p2 = ppool.tile([W, S], f32, tag="p12")
                nc.tensor.matmul(p2[:], w1t[:], h1[:], start=True, stop=True)
                u2 = wpool.tile([W, S], f32, tag="u2")
                h2 = wpool.tile([W, S], f32, tag="h2")
                nc.scalar.activation(u2[:], p2[:], AF.Exp, bias=b1[:])
                nc.scalar.activation(h2[:], u2[:], AF.Ln, bias=1.0)
                # ---- mm3: z = h2^T W2T + b2 -> (S, H*D)
                vf = ppool.tile([S, H * D], f32, tag="vf")
                nc.tensor.matmul(vf[:], ones1[:], b2[:], start=True, stop=False)
                nc.tensor.matmul(vf[:], h2[:], w2t[:], start=False, stop=True)
                # ---- S = sigmoid(-2z) = exp(-ln(1+exp(2z)))
                e2 = wpool.tile([S, H * D], f32, tag="e2")
                l2 = wpool.tile([S, H * D], f32, tag="l2")
                nc.scalar.activation(e2[:], vf[:], AF.Exp, scale=2.0)
                nc.scalar.activation(l2[:], e2[:], AF.Ln, bias=1.0)
                s9v = s9k[:].rearrange("s (h d) -> s h d", d=D9)
                l2v = l2[:].rearrange("s (h d) -> s h d", d=D)
                nc.scalar.activation(s9v[:, :, 0:D], l2v, AF.Exp, scale=-1.0)
                # ---- q[s,h] = sum_d S9 * dX9  (broadcast dx over h)
                m1 = wpool.tile([S, H * D9], f32, tag="m1")
                dxk = dx9[:, bass.ds(ibase + j * D9, D9)]
                dxb = dxk.rearrange("s (o d) -> s o d", o=1)
                m1v = m1[:].rearrange("s (h d) -> s h d", d=D9)
                s9vv = s9k[:].rearrange("s (h d) -> s h d", d=D9)
                in0b, in1b = bass.broadcast_tensor_aps(s9vv, dxb)
                nc.vector.tensor_tensor(m1v, in0b, in1b, ALU.mult)
                q = wpool.tile([S, H], f32, tag="q")
                nc.vector.tensor_reduce(
                    q[:], m1v, axis=mybir.AxisListType.X, op=ALU.add
                )
                # ---- y += q^T
                qt = ppool.tile([H, S], f32, tag="qt")
                nc.tensor.transpose(qt[:], q[:], ident[:])
                nc.vector.tensor_tensor(y[:], y[:], qt[:], ALU.add)
                # ---- readout column
                nc.tensor.matmul(
                    ro_ps[:, j : j + 1], y[:], wlt[:], start=True, stop=True
                )
                if j == UNROLL - 1:
                    nc.vector.tensor_copy(
                        ro_sb[:, bass.ds(iv * UNROLL, UNROLL)], ro_ps[:]
                    )

            # ---- final sigmoid(v + bl) = exp(-ln(1+exp(-v-bl)))
            eo = spool.tile([S, T], f32, tag="eo")
            nc.scalar.activation(eo[:], ro_sb[:], AF.Exp, scale=-1.0,
                                 bias=float(-bl_val))
            nc.scalar.activation(eo[:], eo[:], AF.Ln, bias=1.0)
            nc.scalar.activation(eo[:], eo[:], AF.Exp, scale=-1.0)
            nc.sync.dma_start(d_out[:], eo[:])

    nc.compile()
    return nc


_NC_CACHE = {}
LAST_RESULTS = None


def _get_nc(bl_val):
    key = float(bl_val)
    if key not in _NC_CACHE:
        _NC_CACHE[key] = _build_kernel(key)
    return _NC_CACHE[key]


# ------------------------------------------------------------------- driver
def kernel(ts, cd, cc, cb, ca, Wi0, bi0, Wi1, bi1, Wi2, bi2,
           Wf0, bf0, Wf1, bf1, Wf2, bf2, Wl, bl):
    from concourse.bass_utils import run_bass_kernel_spmd

    f32 = np.float32
    ts, cd, cc, cb, ca = (np.asarray(x, f32) for x in (ts, cd, cc, cb, ca))
    dX9, y0 = _host_precompute(ts, cd, cc, cb, ca, Wi0, bi0, Wi1, bi1, Wi2, bi2)

    Wf0, Wf1, Wf2, Wl = (np.asarray(x, f32) for x in (Wf0, Wf1, Wf2, Wl))
    bf0, bf1, bf2, bl = (np.asarray(x, f32) for x in (bf0, bf1, bf2, bl))

    shared = {
        "w0t": np.ascontiguousarray(Wf0.T),              # (H, W)
        "w1t": np.ascontiguousarray(Wf1.T),              # (W, W)
        "w2t": np.ascontiguousarray(Wf2.T),              # (W, H*D)
        "wlt": np.ascontiguousarray(Wl[0][:, None]),     # (H, 1)
        "b0": np.ascontiguousarray(bf0[:, None]),
        "b1": np.ascontiguousarray(bf1[:, None]),
        "b2": np.ascontiguousarray(bf2[None, :]),
        "ones1": np.ones((1, S), f32),
        "ident": np.eye(S, dtype=f32),
    }

    nc = _get_nc(float(bl[0]))
    in_maps = []
    for c in range(NCORES):
        sl = slice(c * S, (c + 1) * S)
        m = dict(shared)
        m["dx9"] = np.ascontiguousarray(dX9[sl].reshape(S, T * D9))
        m["y0t"] = np.ascontiguousarray(y0[sl].T)        # (H, S)
        in_maps.append(m)

    res = run_bass_kernel_spmd(nc, in_maps, core_ids=list(range(NCORES)))
    global LAST_RESULTS
    LAST_RESULTS = res
    out = np.concatenate([res.results[c]["out"] for c in range(NCORES)], axis=0)
    return out.astype(f32)
